# revision 18
# baseline (speedup 1.0000x reference)
"""Trainium2 Bass kernel for NeuroISNet GNN message passing.

Strategy (8 NeuronCores, one trn2 chip):
  - Batch b -> core pair (2b, 2b+1); each core owns 2048 of 4096 node rows.
  - The dominant einsum msg = x @ m runs in fp8e4 with DoubleRow perf
    mode (2 k-chunks per matmul, 2x PE throughput); x.T stays resident
    in SBUF as fp8 (8MB), halving the startup DMA as well.
  - The msg-MLP runs only on the LOCAL 2048 rows; the per-iteration
    AllGather ships the fp8 MLP output m (not hn), and its rank-ordered
    output blocks land directly in the globally-indexed stationary
    buffer mFq - symmetric across the core pair.
  - LayerNorm scale 1/sqrt(var+eps) is computed entirely on the vector
    engine (bitcast + magic constant + 2 Newton steps), and the affine
    apply runs on the scalar engine as Identity(scale, bias).  This
    keeps every scalar-engine function inside one activation table set
    (sigmoid/tanh/relu/copy/identity) - zero ACT_TABLE_LOAD swaps in
    steady state (the baseline spent 220us on 172 of them).
  - LSTM elementwise math in bf16 (2x DVE throughput); per-rb staggering
    overlaps the bmm (PE) with LSTM/LN chains (Act/DVE) and collectives.
  - Iteration 1 exploits identical initial rows: msg1 = m0 x rowsums,
    a single rank-1 matmul instead of the full bmm.
"""

import numpy as np
import ml_dtypes

import concourse.bass as bass
import concourse.mybir as mybir
import concourse.tile as tile
from concourse import bacc
from concourse.bass_utils import run_bass_kernel_spmd

BF = ml_dtypes.bfloat16
E4 = ml_dtypes.float8_e4m3
bf16 = mybir.dt.bfloat16
fp8 = mybir.dt.float8e4
f32 = mybir.dt.float32
i32 = mybir.dt.int32

B, N, H, ITERS = 4, 4096, 128, 8
EPS = 1e-5
NCORES = 8
R = N // 2              # rows per core
GROUPS = [[0, 1], [2, 3], [4, 5], [6, 7]]
MAGIC = 0x5F3759DF

AF = mybir.ActivationFunctionType
ALU = mybir.AluOpType
DR = mybir.MatmulPerfMode.DoubleRow


def build_module(n_nodes=N, iters=ITERS):
    r = n_nodes // 2            # local rows per core
    kc = n_nodes // 128         # k-chunks (global)
    npair = kc // 2             # DoubleRow chunk pairs
    nrb = max(1, r // 512)      # local 512-row blocks
    rbsz = r // nrb             # 512
    ntt = rbsz // 128           # 128-col tiles per rb

    nc = bacc.Bacc("TRN2", target_bir_lowering=False, debug=False,
                   num_devices=NCORES)

    din = lambda name, shape, dt: nc.dram_tensor(name, shape, dt,
                                                 kind="ExternalInput")
    xt_in = din("xt", [n_nodes, r], fp8)
    h0_in = din("h0", [H, r], bf16)
    rs_in = din("rs", [1, r], bf16)
    m0_in = din("m0", [1, H], bf16)
    b3r_in = din("b3r", [1, H], bf16)
    w1gt_in = din("w1gt", [H, H], bf16)
    w2t_in = din("w2t", [H, H], bf16)
    w3t_in = din("w3t", [H, H], bf16)
    vw1t_in = din("vw1gt", [H, H], bf16)
    vw2t_in = din("vw2t", [H, H], bf16)
    vw3t_in = din("vw3t", [H, 1], bf16)
    wiht_in = din("wiht", [H, 4 * H], bf16)
    whht_in = din("whht", [H, 4 * H], bf16)
    b1c_in = din("b1c", [H, 1], f32)
    b2c_in = din("b2c", [H, 1], f32)
    vb1c_in = din("vb1c", [H, 1], f32)
    vb2c_in = din("vb2c", [H, 1], f32)
    bgc_in = din("bgc", [H, 4], f32)
    ident_in = din("ident", [H, H], bf16)
    ident8_in = din("ident8", [H, H], fp8)

    votes_out = nc.dram_tensor("votes", [1, r], f32, kind="ExternalOutput")

    with tile.TileContext(nc) as tc:
        with tc.tile_pool(name="const", bufs=1) as cp, \
             tc.tile_pool(name="state", bufs=1) as st, \
             tc.tile_pool(name="work", bufs=1) as wk, \
             tc.tile_pool(name="ps", bufs=1, space="PSUM") as ps, \
             tc.tile_pool(name="dram", bufs=1, space="DRAM") as dr:

            # ---- constants ----
            def cload(inp, shape, dt, tag):
                t = cp.tile(shape, dt, tag=tag, name=tag)
                nc.sync.dma_start(t[:], inp[:])
                return t

            w1gt = cload(w1gt_in, [H, H], bf16, "w1gt")
            w2t = cload(w2t_in, [H, H], bf16, "w2t")
            w3t = cload(w3t_in, [H, H], bf16, "w3t")
            vw1gt = cload(vw1t_in, [H, H], bf16, "vw1gt")
            vw2t = cload(vw2t_in, [H, H], bf16, "vw2t")
            vw3t = cload(vw3t_in, [H, 1], bf16, "vw3t")
            wiht = cload(wiht_in, [H, 4 * H], bf16, "wiht")
            whht = cload(whht_in, [H, 4 * H], bf16, "whht")
            b1c = cload(b1c_in, [H, 1], f32, "b1c")
            b2c = cload(b2c_in, [H, 1], f32, "b2c")
            vb1c = cload(vb1c_in, [H, 1], f32, "vb1c")
            vb2c = cload(vb2c_in, [H, 1], f32, "vb2c")
            bgc = cload(bgc_in, [H, 4], f32, "bgc")
            ident = cload(ident_in, [H, H], bf16, "ident")
            ident8 = cload(ident8_in, [H, H], fp8, "ident8")
            rs_sb = cload(rs_in, [1, r], bf16, "rs")
            m0_sb = cload(m0_in, [1, H], bf16, "m0")
            b3r = cload(b3r_in, [1, H], bf16, "b3r")
            mgc = cp.tile([128, ntt], i32, tag="mgc", name="mgc")
            nc.vector.memset(mgc[:], MAGIC)

            # ---- state tiles ----
            h_rb, c_rb = [], []
            for rb in range(nrb):
                ht = st.tile([H, rbsz], bf16, tag=f"h{rb}", name=f"h{rb}")
                nc.sync.dma_start(ht[:], h0_in[:, rb * rbsz:(rb + 1) * rbsz])
                ct = st.tile([H, rbsz], bf16, tag=f"c{rb}", name=f"c{rb}")
                nc.vector.memset(ct[:], 0.0)
                h_rb.append(ht)
                c_rb.append(ct)
            hnL = st.tile([128, r], fp8, tag="hnL", name="hnL")
            # global-chunk-indexed m (bmm stationary), fp8 [128, kc, H]
            mFq = st.tile([128, kc, H], fp8, tag="mFq", name="mFq")

            # ---- resident x^T chunk pairs (global order) ----
            xt = []
            for p in range(npair):
                t = st.tile([128, 2, r], fp8, tag=f"xt{p}", name=f"xt{p}")
                for i in range(2):
                    nc.sync.dma_start(
                        t[:, i, :],
                        xt_in[(2 * p + i) * 128:(2 * p + i + 1) * 128, :])
                xt.append(t)

            # ---- DRAM bounce buffers for collectives ----
            cc_in = [dr.tile([128, ntt * H], fp8, tag=f"cci{rb}", bufs=2,
                             name=f"cci{rb}") for rb in range(nrb)]
            cc_out = [dr.tile([2 * 128, ntt * H], fp8, tag=f"cco{rb}", bufs=2,
                              name=f"cco{rb}") for rb in range(nrb)]

            def bmm_rb(rb, it):
                """msgT for local rows block rb -> psum tile (returned)."""
                mp = ps.tile([H, rbsz], f32, tag="pacc", bufs=2,
                             name=f"msg_{it}_{rb}")
                sl = slice(rb * rbsz, (rb + 1) * rbsz)
                if it == 1:
                    nc.tensor.matmul(mp[:], m0_sb[:], rs_sb[:, sl],
                                     start=True, stop=True)
                    return mp
                nc.tensor.matmul(mp[:], b3r[:], rs_sb[:, sl],
                                 start=True, stop=False)
                # pair order: by source-gather rb' so the last-gathered
                # slots are consumed last
                for rbp in range(nrb):
                    for q in (2 * rbp, 2 * rbp + 1,
                              npair // 2 + 2 * rbp, npair // 2 + 2 * rbp + 1):
                        nc.tensor.matmul(
                            mp[:], mFq[:, 2 * q:2 * q + 2, :],
                            xt[q][:, :, sl],
                            start=False,
                            stop=(rbp == nrb - 1 and
                                  q == npair // 2 + 2 * rbp + 1),
                            perf_mode=DR)
                return mp

            def lstm_rb(rb, mp, it):
                """gates + c/h update for block rb from msg psum tile."""
                msgb = wk.tile([H, rbsz], bf16, tag="msgb", bufs=2,
                               name=f"msgb_{it}_{rb}")
                nc.scalar.activation(msgb[:], mp[:], AF.Copy)
                gact = []
                for g in range(4):
                    gp = ps.tile([H, rbsz], f32, tag="pb", bufs=3,
                                 name=f"gp_{it}_{rb}_{g}")
                    nc.tensor.matmul(gp[:], wiht[:, g * H:(g + 1) * H],
                                     msgb[:], start=True, stop=False)
                    nc.tensor.matmul(gp[:], whht[:, g * H:(g + 1) * H],
                                     h_rb[rb][:], start=False, stop=True)
                    ga = wk.tile([H, rbsz], bf16, tag=f"ga{g}", bufs=2,
                                 name=f"ga_{it}_{rb}_{g}")
                    nc.scalar.activation(
                        ga[:], gp[:],
                        AF.Tanh if g == 2 else AF.Sigmoid,
                        bias=bgc[:, g:g + 1])
                    gact.append(ga)
                si, sf, tg, so = gact
                t1 = wk.tile([H, rbsz], bf16, tag="t1", bufs=2,
                             name=f"t1_{it}_{rb}")
                nc.vector.tensor_tensor(t1[:], sf[:], c_rb[rb][:], ALU.mult)
                t2 = wk.tile([H, rbsz], bf16, tag="t2", bufs=2,
                             name=f"t2_{it}_{rb}")
                nc.vector.tensor_tensor(t2[:], si[:], tg[:], ALU.mult)
                nc.vector.tensor_tensor(c_rb[rb][:], t1[:], t2[:], ALU.add)
                tnc = wk.tile([H, rbsz], bf16, tag="tnc", bufs=2,
                              name=f"tnc_{it}_{rb}")
                nc.scalar.activation(tnc[:], c_rb[rb][:], AF.Tanh)
                nc.vector.tensor_tensor(h_rb[rb][:], so[:], tnc[:], ALU.mult)

            def ln_front_rb(rb, it):
                """transpose + stats + DVE-only rsqrt + affine apply."""
                trp = ps.tile([128, ntt, 128], bf16, tag="ptr", bufs=2,
                              name=f"trp_{it}_{rb}")
                mvb = wk.tile([128, ntt, 2], f32, tag="mvb", bufs=2,
                              name=f"mvb_{it}_{rb}")
                for t in range(ntt):
                    nc.tensor.transpose(
                        trp[:, t, :], h_rb[rb][:, t * 128:(t + 1) * 128],
                        ident[:])
                    stt = wk.tile([128, 6], f32, tag="st6", bufs=3,
                                  name=f"st_{it}_{rb}_{t}")
                    nc.vector.bn_stats(stt[:], trp[:, t, :])
                    nc.vector.bn_aggr(mvb[:, t, :], stt[:])
                # s = rsqrt(var+eps) via bitcast magic + 2 Newton (DVE only)
                vpe = wk.tile([128, ntt], f32, tag="vpe", bufs=2,
                              name=f"vpe_{it}_{rb}")
                nc.vector.tensor_scalar_add(vpe[:], mvb[:, :, 1], EPS)
                shi = wk.tile([128, ntt], i32, tag="shi", bufs=2,
                              name=f"shi_{it}_{rb}")
                nc.vector.tensor_scalar(shi[:], vpe[:].bitcast(i32), 1, None,
                                        op0=ALU.logical_shift_right)
                y0i = wk.tile([128, ntt], i32, tag="y0i", bufs=2,
                              name=f"y0i_{it}_{rb}")
                nc.vector.scalar_tensor_tensor(y0i[:], mgc[:], 0, shi[:],
                                               op0=ALU.bypass,
                                               op1=ALU.subtract)
                y = y0i[:].bitcast(f32)
                aa = wk.tile([128, ntt], f32, tag="aa", bufs=2,
                             name=f"aa_{it}_{rb}")
                bb = wk.tile([128, ntt], f32, tag="bb", bufs=2,
                             name=f"bb_{it}_{rb}")
                wt2 = wk.tile([128, ntt], f32, tag="wt2", bufs=2,
                              name=f"wt2_{it}_{rb}")
                sss = wk.tile([128, ntt], f32, tag="sss", bufs=2,
                              name=f"sss_{it}_{rb}")
                for newt in range(2):
                    nc.vector.tensor_tensor(aa[:], y, y, ALU.mult)
                    nc.vector.tensor_tensor(bb[:], vpe[:], aa[:], ALU.mult)
                    nc.vector.tensor_scalar(wt2[:], bb[:], -0.5, 1.5,
                                            op0=ALU.mult, op1=ALU.add)
                    nc.vector.tensor_tensor(sss[:], y, wt2[:], ALU.mult)
                    y = sss[:]
                ngm = wk.tile([128, ntt], f32, tag="ngm", bufs=2,
                              name=f"ngm_{it}_{rb}")
                nc.vector.scalar_tensor_tensor(ngm[:], mvb[:, :, 0], -1.0,
                                               sss[:], op0=ALU.mult,
                                               op1=ALU.mult)
                hnr = wk.tile([128, ntt, 128], bf16, tag="hnr", bufs=2,
                              name=f"hnr_{it}_{rb}")
                for t in range(ntt):
                    nc.scalar.activation(hnr[:, t, :], trp[:, t, :],
                                         AF.Identity,
                                         bias=ngm[:, t:t + 1],
                                         scale=sss[:, t:t + 1])
                return hnr

            def ln_back_rb(rb, it, hnr):
                """transpose back -> hnL slice (fp8)."""
                hnp = ps.tile([128, ntt, 128], bf16, tag="ptq", bufs=1,
                              name=f"hnp_{it}_{rb}")
                for t in range(ntt):
                    nc.tensor.transpose(hnp[:, t, :], hnr[:, t, :], ident[:])
                    nc.vector.tensor_copy(
                        hnL[:, rb * rbsz + t * 128:rb * rbsz + (t + 1) * 128],
                        hnp[:, t, :])

            def mlp_rb(rb, it, gather):
                """local msg MLP on hnL block rb -> staging -> gather to mFq."""
                sl = slice(rb * rbsz, (rb + 1) * rbsz)
                m1p = ps.tile([H, rbsz], f32, tag="pb", bufs=3,
                              name=f"m1p_{it}_{rb}")
                nc.tensor.matmul(m1p[:], w1gt[:], hnL[:, sl],
                                 start=True, stop=True)
                m1s = wk.tile([H, rbsz], bf16, tag="m1s", bufs=2,
                              name=f"m1s_{it}_{rb}")
                nc.scalar.activation(m1s[:], m1p[:], AF.Relu, bias=b1c[:])
                m2p = ps.tile([H, rbsz], f32, tag="pb", bufs=3,
                              name=f"m2p_{it}_{rb}")
                nc.tensor.matmul(m2p[:], w2t[:], m1s[:], start=True, stop=True)
                m2s = wk.tile([H, rbsz], bf16, tag="m2s", bufs=2,
                              name=f"m2s_{it}_{rb}")
                nc.scalar.activation(m2s[:], m2p[:], AF.Relu, bias=b2c[:])
                mloc = wk.tile([128, ntt, H], fp8, tag="mloc", bufs=2,
                               name=f"mloc_{it}_{rb}")
                m3p = ps.tile([H, rbsz], f32, tag="pb", bufs=3,
                              name=f"m3p_{it}_{rb}")
                for t in range(ntt):
                    nc.tensor.matmul(m3p[:, t * H:(t + 1) * H],
                                     m2s[:, t * 128:(t + 1) * 128],
                                     w3t[:], start=True, stop=True)
                    nc.scalar.activation(mloc[:, t, :],
                                         m3p[:, t * H:(t + 1) * H], AF.Copy)
                if gather:
                    nc.sync.dma_start(cc_in[rb][:], mloc[:])
                    nc.gpsimd.collective_compute(
                        "AllGather", ALU.bypass, replica_groups=GROUPS,
                        ins=[cc_in[rb][:].opt()], outs=[cc_out[rb][:].opt()])
                    # rank-ordered output blocks -> global chunk slots
                    nc.sync.dma_start(mFq[:, ntt * rb:ntt * (rb + 1), :],
                                      cc_out[rb][0:128, :])
                    nc.sync.dma_start(
                        mFq[:, kc // 2 + ntt * rb:kc // 2 + ntt * (rb + 1), :],
                        cc_out[rb][128:256, :])

            # ================= main loop =================
            # Slot schedule keeps the in-order PE stream fed: the cheap
            # PE pieces of lstm/ln interleave between bmm blocks, and
            # ln_back/mlp are deferred until their Act/DVE deps are done.
            SLOTS = [("bmm", 0), ("bmm", 1), ("lstm", 0), ("bmm", 2),
                     ("lstm", 1), ("lnf", 0), ("bmm", 3), ("lstm", 2),
                     ("lnf", 1), ("fin", 0), ("lstm", 3), ("lnf", 2),
                     ("fin", 1), ("lnf", 3), ("fin", 2), ("fin", 3)]
            for it in range(1, iters + 1):
                mps, lnst = {}, {}
                for ph, rb in SLOTS:
                    if ph == "bmm":
                        mps[rb] = bmm_rb(rb, it)
                    elif ph == "lstm":
                        lstm_rb(rb, mps.pop(rb), it)
                    elif ph == "lnf":
                        lnst[rb] = ln_front_rb(rb, it)
                    else:
                        ln_back_rb(rb, it, lnst.pop(rb))
                        if it < iters:
                            mlp_rb(rb, it, gather=True)

            # ================= vote =================
            for rb in range(nrb):
                sl = slice(rb * rbsz, (rb + 1) * rbsz)
                v1p = ps.tile([H, rbsz], f32, tag="pb", bufs=3,
                              name=f"v1p_{rb}")
                nc.tensor.matmul(v1p[:], vw1gt[:], hnL[:, sl],
                                 start=True, stop=True)
                v1s = wk.tile([H, rbsz], bf16, tag="m1s", bufs=2,
                              name=f"v1s_{rb}")
                nc.scalar.activation(v1s[:], v1p[:], AF.Relu, bias=vb1c[:])
                v2p = ps.tile([H, rbsz], f32, tag="pb", bufs=3,
                              name=f"v2p_{rb}")
                nc.tensor.matmul(v2p[:], vw2t[:], v1s[:], start=True, stop=True)
                v2s = wk.tile([H, rbsz], bf16, tag="m2s", bufs=2,
                              name=f"v2s_{rb}")
                nc.scalar.activation(v2s[:], v2p[:], AF.Relu, bias=vb2c[:])
                v3t = ps.tile([H, rbsz], f32, tag="pb", bufs=3,
                              name=f"v3t_{rb}")
                nc.tensor.matmul(v3t[0:1, :], vw3t[:], v2s[:],
                                 start=True, stop=True)
                vos = wk.tile([1, rbsz], f32, tag="vos", bufs=2,
                              name=f"vos_{rb}")
                nc.scalar.activation(vos[:], v3t[0:1, :], AF.Copy)
                nc.sync.dma_start(votes_out[:, sl], vos[:])

    nc.compile()
    return nc


_NC_CACHE = {}


def _get_module():
    key = (N, ITERS)
    if key not in _NC_CACHE:
        _NC_CACHE[key] = build_module(N, ITERS)
    return _NC_CACHE[key]


def _host_prep(inputs):
    """Fold weights, run init MLP, build per-core in_maps."""
    g = lambda s: np.asarray(inputs[s], np.float32)
    x = g("x")
    k, n = g("k"), g("n")

    nk = np.stack([k, n], 1)
    a = np.maximum(nk @ g("init_w1").T + g("init_b1"), 0)
    a = np.maximum(a @ g("init_w2").T + g("init_b2"), 0)
    init0 = a @ g("init_w3").T + g("init_b3")          # [B, H]

    ln_g, ln_b = g("ln_g"), g("ln_b")
    mu0 = init0.mean(1, keepdims=True)
    var0 = init0.var(1, keepdims=True)
    embed0 = (init0 - mu0) / np.sqrt(var0 + EPS) * ln_g + ln_b
    t = np.maximum(embed0 @ g("msg_w1").T + g("msg_b1"), 0)
    t = np.maximum(t @ g("msg_w2").T + g("msg_b2"), 0)
    m0eff = t @ g("msg_w3").T + g("msg_b3")            # [B, H]

    com = {
        "w1gt": (g("msg_w1") * ln_g[None, :]).T.astype(BF),
        "w2t": g("msg_w2").T.astype(BF),
        "w3t": g("msg_w3").T.astype(BF),
        "vw1gt": (g("vote_w1") * ln_g[None, :]).T.astype(BF),
        "vw2t": g("vote_w2").T.astype(BF),
        "vw3t": g("vote_w3").T.astype(BF),              # [H, 1]
        "wiht": g("lstm_wih").T.astype(BF),
        "whht": g("lstm_whh").T.astype(BF),
        "b1c": (g("msg_w1") @ ln_b + g("msg_b1")).reshape(H, 1).astype(np.float32),
        "b2c": g("msg_b2").reshape(H, 1).astype(np.float32),
        "vb1c": (g("vote_w1") @ ln_b + g("vote_b1")).reshape(H, 1).astype(np.float32),
        "vb2c": g("vote_b2").reshape(H, 1).astype(np.float32),
        "bgc": (g("lstm_bih") + g("lstm_bhh")).reshape(4, H).T.astype(np.float32).copy(),
        "b3r": g("msg_b3").reshape(1, H).astype(BF),
        "ident": np.eye(H, dtype=BF),
        "ident8": np.eye(H, dtype=E4),
    }

    in_maps = []
    for core in range(NCORES):
        b = core // 2
        r0 = (core % 2) * R
        xs8 = np.ascontiguousarray(x[b][r0:r0 + R, :].T).astype(E4)  # [N, R]
        m = dict(com)
        m["xt"] = xs8
        m["rs"] = xs8.astype(np.float32).sum(0).reshape(1, R).astype(BF)
        m["h0"] = np.ascontiguousarray(
            np.broadcast_to(init0[b][:, None], (H, R))).astype(BF)
        m["m0"] = m0eff[b].reshape(1, H).astype(BF)
        in_maps.append(m)
    return in_maps


def kernel(**inputs):
    nc = _get_module()
    in_maps = _host_prep(inputs)
    res = run_bass_kernel_spmd(nc, in_maps, core_ids=list(range(NCORES)))
    mask = np.asarray(inputs["mask"], np.float64)
    vb3 = float(np.asarray(inputs["vote_b3"], np.float64).reshape(-1)[0])
    out = np.zeros(B, np.float32)
    for b in range(B):
        votes = np.concatenate([
            res.results[2 * b]["votes"].reshape(-1),
            res.results[2 * b + 1]["votes"].reshape(-1),
        ]).astype(np.float64) + vb3
        s = float((votes * mask[b]).sum())
        out[b] = 1.0 / (1.0 + np.exp(-s))
    return out
